# revision 12
# baseline (speedup 1.0000x reference)
"""DualContextAttention Trainium2 kernel (v2).

Sharding: 8 cores = 4 batches x 2 query-halves. Each core (b, s) runs
attention for batch b over query positions n in [2048*s, 2048*(s+1))
plus the d1 gate (y = feat * gate1) and the per-half pooled sums
(phase A). Phase B consumes feat + the gathered y halo rows + global
pooled sums and runs l1, the 3x3 conv stack (l2 in fp8 DoubleRow),
l3, the SE branch and the final gating.

Numerics (validated against the reference in numpy emulation):
- Q/K/V projections and the energy matmul run in f32r (tf32-like);
  fp8/bf16 on the q/k path overshoots the 2e-2 error budget.
- k needs no bias: a per-query additive constant cancels in softmax.
  bv is folded in after the PE transposes (per-partition add) and into
  d1's bias on the host.
- exp() output, attention probabilities, V, feat, y, and all phase B
  activations are bf16 (DVE 2x/4x modes + half DMA traffic).
- l2 (the 3x3 conv) runs in fp8e4m3 DoubleRow: 2 input-channel chunks
  contract per instruction at 0.5 cycles/row.
- Softmax max-subtraction is unnecessary: |energy| << 88 so fp32/bf16
  exp cannot overflow, and exp scale cancels after normalization.
- gate1 sigmoid is computed as relu(tanh(z/2))*0.5+0.5 so phase A's ACT
  stays on the exp_and_others table set (relu commutes with monotone
  tanh through 0).
"""

import numpy as np

import concourse.bass as bass
import concourse.tile as tile
from concourse import mybir
from concourse.alu_op_type import AluOpType
from concourse.bass_utils import run_bass_kernel_spmd
from bass_rust import AxisListType

F32 = mybir.dt.float32
F32R = mybir.dt.float32r
BF16 = mybir.dt.bfloat16
FP8 = mybir.dt.float8e4
DR = mybir.MatmulPerfMode.DoubleRow
ACT = mybir.ActivationFunctionType

B, C, C2, H, W = 4, 256, 128, 64, 64
N = H * W          # 4096
NH = N // 2        # 2048 query positions per core
NG = NH // 512     # 512-wide query groups per core
NHALO = 34 * 64    # l1 positions incl. one halo row each side
EPS = 1e-5
NCORES = 8
VW = C + 1         # PV output width: 256 channels + denominator column


def _split_multi_waits(nc, max_waits=1):
    """walrus in this container rejects instructions carrying more than one
    sync-wait; hoist extras onto preceding same-engine NoOps."""
    ctr = 0
    for f in nc.m.functions:
        for bb in f.blocks:
            insts = bb.instructions
            out = []
            changed = False
            for inst in insts:
                si = inst.sync_info
                if (
                    si is not None
                    and si.on_wait is not None
                    and len(si.on_wait) > max_waits
                ):
                    waits = list(si.on_wait)
                    for w in waits[:-max_waits]:
                        out.append(
                            mybir.InstNoOp(
                                name=f"wsplit-{ctr}",
                                engine=inst.engine,
                                sync_info=mybir.SyncInfo(on_wait=[w], on_update=[]),
                            )
                        )
                        ctr += 1
                    inst.sync_info = mybir.SyncInfo(
                        on_wait=waits[-max_waits:], on_update=list(si.on_update)
                    )
                    changed = True
                out.append(inst)
            if changed:
                bb.instructions = out
    return ctr


def _mm(nc, out, lhsT, rhs, start, stop, perf_mode=None):
    nc.tensor.matmul(out, lhsT, rhs, start=start, stop=stop, perf_mode=perf_mode)


# ---------------------------------------------------------------- phase A


def _build_phase_a():
    nc = bass.Bass()
    xb = nc.dram_tensor("xb", [2, 128, N], F32R, kind="ExternalInput")
    # f32r weights: wq [2ci,128], wk [2ci,128], wv [2ci,256]
    wf = nc.dram_tensor("wf", [128, 1024], F32R, kind="ExternalInput")
    # bf16 weights: d1w [2ci,2cb,128], identity [128]
    wb = nc.dram_tensor("wb", [128, 640], BF16, kind="ExternalInput")
    # f32 per-partition bias columns: bq 1, bv 2, d1bh 2
    wc = nc.dram_tensor("wc", [128, 5], F32, kind="ExternalInput")

    feat_d = nc.dram_tensor("feat", [2, 128, NH], BF16, kind="ExternalOutput")
    y_d = nc.dram_tensor("y", [2, 128, NH], BF16, kind="ExternalOutput")
    ysum_d = nc.dram_tensor("ysum", [128, 2], F32, kind="ExternalOutput")

    with tile.TileContext(nc) as tc:
        with (
            tc.tile_pool(name="wp", bufs=1) as wp,
            tc.tile_pool(name="kqv", bufs=1) as kqv,
            tc.tile_pool(name="outp", bufs=1) as outp,
            tc.tile_pool(name="ps", bufs=1, space="PSUM") as ps,
        ):
            wf_sb = wp.tile([128, 1024], F32R)
            wb_sb = wp.tile([128, 640], BF16)
            wc_sb = wp.tile([128, 5], F32)
            # bias columns + qk weights first so the projections start asap;
            # wv rides the scalar queue behind the first x chunk, d1/identity
            # weights (not needed until the first norm tail) go last.
            nc.sync.dma_start(out=wc_sb, in_=wc[:, :])
            nc.sync.dma_start(out=wf_sb[:, 0:512], in_=wf[:, 0:512])
            wq_sb = wf_sb[:, 0:256].rearrange("p (a m) -> p a m", a=2)
            wk_sb = wf_sb[:, 256:512].rearrange("p (a m) -> p a m", a=2)
            wv_sb = wf_sb[:, 512:1024].rearrange("p (a m) -> p a m", a=2)
            d1w_sb = wb_sb[:, 0:512].rearrange("p (a c m) -> p a c m", a=2, c=2)
            ident_sb = wb_sb[:, 512:640]
            bq_col = wc_sb[:, 0:1]
            bv_col = wc_sb[:, 1:3]
            d1bh_col = wc_sb[:, 3:5]

            # act-table warmup (exp_and_others) overlapped with input DMA
            warm_z = wp.tile([1, 1], F32)
            nc.vector.memset(warm_z, 0.0)
            warm = wp.tile([1, 1], F32)
            nc.scalar.activation(warm, warm_z, ACT.Exp)

            k_sb = kqv.tile([128, N], F32R)
            q_sb = kqv.tile([128, NH], F32R)
            vT_sb = kqv.tile([128, 32, VW], BF16)
            # denominator column: ones
            nc.vector.memset(vT_sb[:, :, C : C + 1], 1.0)

            feat_sb = outp.tile([128, 2, NH], BF16)
            y_sb = outp.tile([128, 2, NH], BF16)
            ys_parts = outp.tile([128, 2, NG + 1], F32)
            ys_sb = outp.tile([128, 2], F32)

            # ---- load x + projections (x freed after this block).
            # x streams in 8 fine chunks; K/Q/V matmuls for each chunk issue
            # as soon as its DMA lands so the PE ramps immediately.
            with tc.tile_pool(name="xp", bufs=1) as xp:
                NC8 = N // 8
                xb_t = [
                    xp.tile([128, 2, NC8], F32R, name=f"xbc{c8}")
                    for c8 in range(8)
                ]
                for c8 in range(8):
                    for a in range(2):
                        eng = nc.sync if a == 0 else nc.scalar
                        eng.dma_start(
                            out=xb_t[c8][:, a, :],
                            in_=xb[a][:, bass.ts(c8, NC8)],
                        )
                    if c8 == 0:
                        nc.scalar.dma_start(out=wf_sb[:, 512:1024],
                                            in_=wf[:, 512:1024])
                    elif c8 == 1:
                        nc.scalar.dma_start(out=wb_sb, in_=wb[:, :])

                def xb_sl(a, lo, width):
                    c8 = lo // NC8
                    assert lo + width <= (c8 + 1) * NC8
                    return xb_t[c8][:, a, lo - c8 * NC8 : lo - c8 * NC8 + width]

                slot_tags = [("pv0", 1), ("pv1", 1), ("pv2", 1), ("pv3", 1),
                             ("e", 2), ("e", 2)]
                slot_i = [0]

                def _ptile(shape):
                    tg, bf = slot_tags[slot_i[0] % 6]
                    slot_i[0] += 1
                    return ps.tile(shape, F32, tag=tg, bufs=bf,
                                   name=f"proj{slot_i[0]}")

                for j in range(8):
                    pk = _ptile([128, 512])
                    _mm(nc, pk, wk_sb[:, 0, :], xb_sl(0, j * 512, 512),
                        True, False)
                    _mm(nc, pk, wk_sb[:, 1, :], xb_sl(1, j * 512, 512),
                        False, True)
                    if j % 2 == 0:
                        nc.scalar.activation(k_sb[:, bass.ts(j, 512)], pk,
                                             ACT.Copy)
                    else:
                        nc.vector.tensor_copy(k_sb[:, bass.ts(j, 512)], pk)
                    if j < 4:
                        pq = _ptile([128, 512])
                        _mm(nc, pq, wq_sb[:, 0, :], xb_sl(0, j * 512, 512),
                            True, False)
                        _mm(nc, pq, wq_sb[:, 1, :], xb_sl(1, j * 512, 512),
                            False, True)
                        nc.vector.tensor_scalar_add(q_sb[:, bass.ts(j, 512)],
                                                    pq, bq_col)
                    for mb in range(4 * j, 4 * j + 4):
                        pv = _ptile([128, C])
                        _mm(nc, pv, xb_sl(0, mb * 128, 128), wv_sb[:, 0, :],
                            True, False)
                        _mm(nc, pv, xb_sl(1, mb * 128, 128), wv_sb[:, 1, :],
                            False, True)
                        if mb % 2 == 0:
                            nc.vector.tensor_copy(vT_sb[:, mb, 0:C], pv)
                        else:
                            nc.scalar.activation(vT_sb[:, mb, 0:C], pv,
                                                 ACT.Copy)

            attn_ctx = tc.tile_pool(name="attn", bufs=1)
            small_ctx = tc.tile_pool(name="small", bufs=3)
            tmp_ctx = tc.tile_pool(name="tmp", bufs=3)
            attn = attn_ctx.__enter__()
            small = small_ctx.__enter__()
            tmp = tmp_ctx.__enter__()

            # ---- per 512-wide query group: attention + gate,
            # software-pipelined one group deep.
            state = {}

            def emit_energy(g):
                gsl = bass.ts(g, 512)
                parts = [
                    attn.tile([128, 2, 512], BF16, tag=f"at{mp}", bufs=2,
                              name=f"at{mp}_{g}")
                    for mp in range(16)
                ]
                for mp in range(16):
                    pe2 = ps.tile([128, 2, 512], F32, tag="e", bufs=2)
                    _mm(nc, pe2[:, 0, :], k_sb[:, bass.ts(2 * mp, 128)],
                        q_sb[:, gsl], True, True)
                    _mm(nc, pe2[:, 1, :], k_sb[:, bass.ts(2 * mp + 1, 128)],
                        q_sb[:, gsl], True, True)
                    nc.scalar.activation(parts[mp], pe2, ACT.Exp)
                state[g] = {"parts": parts}

            def at_chunk(st, mb):
                return st["parts"][mb // 2][:, mb % 2, :]

            def emit_pv(g, nb_outer=False):
                # PV transposed: attn chunk stationary, vT (augmented with a
                # ones column) moving; output column C is the softmax
                # denominator for the 128 queries of the block. For the last
                # group nb runs outer so each query block's normalization can
                # start while the next block is still accumulating.
                st = state[g]
                pvp = [
                    ps.tile([128, VW], F32, tag=f"pv{nb}", bufs=1,
                            name=f"pv{nb}_{g}")
                    for nb in range(4)
                ]
                st["pvp"] = pvp
                order = (
                    [(mb, nb) for nb in range(4) for mb in range(32)]
                    if nb_outer else
                    [(mb, nb) for mb in range(32) for nb in range(4)]
                )
                for mb, nb in order:
                    lhsT = at_chunk(st, mb)[:, bass.ts(nb, 128)]
                    _mm(nc, pvp[nb], lhsT, vT_sb[:, mb, :],
                        mb == 0, mb == 31)
                    if nb_outer and mb == 31:
                        emit_norm_nb(g, nb)
                        if nb == 1:
                            emit_d1_tail(g, 0, 256, NG - 1)
                        elif nb == 3:
                            emit_d1_tail(g, 256, 256, NG)

            def emit_norm_nb(g, nb):
                st = state[g]
                pvp = st["pvp"][nb]
                rc = small.tile([128, 1], F32, tag="recip")
                nc.vector.reciprocal(rc, pvp[:, C : C + 1])
                ftT = small.tile([128, C], BF16, tag="ftT")
                nc.vector.tensor_scalar_mul(ftT, pvp[:, 0:C], rc)
                for cb in range(2):
                    trp = ps.tile([128, 128], BF16, tag=f"pv{nb}",
                                  bufs=1, name=f"trp{nb}_{cb}_{g}")
                    nc.tensor.transpose(
                        trp, ftT[:, bass.ts(cb, 128)], ident_sb
                    )
                    dst = feat_sb[:, cb,
                                  g * 512 + nb * 128 : g * 512 + (nb + 1) * 128]
                    bvc = bv_col[:, cb : cb + 1]
                    nc.vector.tensor_scalar_add(dst, trp, bvc)

            def emit_d1_tail(g, h0, w, slot):
                sl = slice(g * 512 + h0, g * 512 + h0 + w)
                for cb in range(2):
                    pz = ps.tile([128, 512], F32, tag="e", bufs=2)
                    pzw = pz[:, 0:w]
                    _mm(nc, pzw, d1w_sb[:, 0, cb, :], feat_sb[:, 0, sl],
                        True, False)
                    _mm(nc, pzw, d1w_sb[:, 1, cb, :], feat_sb[:, 1, sl],
                        False, True)
                    # gate1 = sigmoid(relu(z)) = relu(tanh(z/2))*0.5 + 0.5
                    th = tmp.tile([128, 512], BF16, tag="th")
                    thw = th[:, 0:w]
                    nc.scalar.activation(thw, pzw, ACT.Tanh, scale=0.5,
                                         bias=d1bh_col[:, cb : cb + 1])
                    gt = tmp.tile([128, 512], BF16, tag="g")
                    gtw = gt[:, 0:w]
                    nc.vector.tensor_scalar(
                        gtw, thw, 0.0, 0.5, AluOpType.max, AluOpType.mult
                    )
                    g2 = tmp.tile([128, 512], BF16, tag="g2")
                    g2w = g2[:, 0:w]
                    nc.vector.tensor_scalar_add(g2w, gtw, 0.5)
                    yt = y_sb[:, cb, sl]
                    nc.vector.tensor_tensor(
                        yt, g2w, feat_sb[:, cb, sl], AluOpType.mult
                    )
                    nc.vector.reduce_sum(
                        ys_parts[:, cb, slot : slot + 1], yt, axis=AxisListType.X
                    )
                    nc.sync.dma_start(out=y_d[cb][:, sl], in_=yt)

            def emit_norm_tail(g, skip_norm=False):
                gsl = bass.ts(g, 512)
                if not skip_norm:
                    for nb in range(4):
                        emit_norm_nb(g, nb)
                    emit_d1_tail(g, 0, 512, g)
                for a in range(2):
                    nc.sync.dma_start(
                        out=feat_d[a][:, gsl], in_=feat_sb[:, a, gsl]
                    )
                del state[g]

            emit_energy(0)
            for g in range(NG):
                if g > 0:
                    emit_norm_tail(g - 1)
                if g + 1 < NG:
                    emit_energy(g + 1)
                emit_pv(g, nb_outer=(g == NG - 1))
            emit_norm_tail(NG - 1, skip_norm=True)

            for cb in range(2):
                nc.vector.reduce_sum(
                    ys_sb[:, cb : cb + 1], ys_parts[:, cb, :], axis=AxisListType.X
                )
            nc.sync.dma_start(out=ysum_d[:, :], in_=ys_sb)

            tmp_ctx.__exit__(None, None, None)
            small_ctx.__exit__(None, None, None)
            attn_ctx.__exit__(None, None, None)

    _split_multi_waits(nc)
    return nc


# ---------------------------------------------------------------- phase B


def _build_phase_b():
    nc = bass.Bass()
    feat = nc.dram_tensor("feat", [2, 128, NH], BF16, kind="ExternalInput")
    yh = nc.dram_tensor("yh", [2, 128, NHALO], BF16, kind="ExternalInput")
    yss = nc.dram_tensor("yss", [128, 4], F32, kind="ExternalInput")
    # per-core boundary-row flags: col 0 scales pad row 0, col 1 pad row 33
    # (the reference zero-pads the l1 OUTPUT at the image boundary, so the
    # halo row on the image edge must be zeroed, not l1(0)=relu(l1b))
    bm = nc.dram_tensor("bm", [128, 2], F32, kind="ExternalInput")
    # l2 weights fp8 DoubleRow pack: [ci_lo, pass(hi/lo), tap(9), cb(2),
    # ci_hi(2), co(128)]; pass 1 is the fp8 residual of the bn-folded weight
    w8 = nc.dram_tensor("w8", [128, 2 * 9 * 2 * 2 * 128], FP8,
                        kind="ExternalInput")
    # bf16: l1w [2ci,2cb,128], l3w [2ci,2cb,128]
    wb = nc.dram_tensor("wb", [128, 1024], BF16, kind="ExternalInput")
    # f32: l1b 2, l2b 2, l3b 2, r1w [2,128], r1b 1, r2w [2cb*128], r2b 2
    wf = nc.dram_tensor("wf", [128, 521], F32, kind="ExternalInput")
    out_d = nc.dram_tensor("out", [2, 128, NH], BF16, kind="ExternalOutput")

    with tile.TileContext(nc) as tc:
        with (
            tc.tile_pool(name="wp", bufs=1) as wp,
            tc.tile_pool(name="act", bufs=1) as actp,
            tc.tile_pool(name="tmp", bufs=3) as tmp,
            tc.tile_pool(name="ps", bufs=2, space="PSUM") as ps,
        ):
            w8_sb = wp.tile([128, 2 * 9 * 2 * 2 * 128], FP8)
            wb_sb = wp.tile([128, 1024], BF16)
            wf_sb = wp.tile([128, 521], F32)
            # DMA engines share one bandwidth pool, so order = priority:
            # l1 deps (wb, wf, bm, yh) first, then w8 for l2, feat last
            # (only the final gating reads it).
            nc.sync.dma_start(out=wb_sb, in_=wb[:, :])
            nc.sync.dma_start(out=wf_sb, in_=wf[:, :])
            l2w = w8_sb.rearrange("p (s t c h m) -> p s t c h m",
                                  s=2, t=9, c=2, h=2)
            l1w = wb_sb[:, 0:512].rearrange("p (a c m) -> p a c m", a=2, c=2)
            l3w = wb_sb[:, 512:1024].rearrange("p (a c m) -> p a c m", a=2, c=2)
            l1b = wf_sb[:, 0:2]
            l2b = wf_sb[:, 2:4]
            l3b = wf_sb[:, 4:6]
            r1w = wf_sb[:, 6:262].rearrange("p (a m) -> p a m", a=2)
            r1b = wf_sb[:, 262:263]
            r2w = wf_sb[:, 263:519]
            r2b = wf_sb[:, 519:521]

            warm_z = wp.tile([1, 1], F32)
            nc.vector.memset(warm_z, 0.0)
            warm = wp.tile([1, 1], F32)
            nc.scalar.activation(warm, warm_z, ACT.Sigmoid)

            feat_sb = actp.tile([128, 2, NH], BF16)
            yh_sb = actp.tile([128, 2, NHALO], BF16)
            pad_sb = actp.tile([128, 2, 34, 66], FP8)
            yss_sb = actp.tile([128, 4], F32)
            pooled = actp.tile([128, 2], F32)
            yr1_sb = actp.tile([128, 1], F32)
            yr_sb = actp.tile([128, 2], F32)

            bm_sb = actp.tile([128, 2], F32)
            nc.sync.dma_start(out=bm_sb, in_=bm[:, :])
            # yh in two row-halves per ci chunk so l1 row-chunk 0 starts
            # as soon as the first halves land
            for r0, r1 in ((0, 17), (17, 34)):
                for a in range(2):
                    nc.scalar.dma_start(
                        out=yh_sb[:, a, r0 * 64 : r1 * 64],
                        in_=yh[a][:, r0 * 64 : r1 * 64],
                    )
            # w8 split per residual pass (hi first) so the hi taps of l2
            # can begin before the lo pack arrives
            HW8 = 9 * 2 * 2 * 128
            nc.gpsimd.dma_start(out=w8_sb[:, 0:HW8], in_=w8[:, 0:HW8])
            nc.gpsimd.dma_start(out=w8_sb[:, HW8:], in_=w8[:, HW8:])
            nc.sync.dma_start(out=yss_sb, in_=yss[:, :])
            for j in range(4):
                sl = bass.ts(j, 512)
                nc.sync.dma_start(
                    out=feat_sb[:, :, sl],
                    in_=feat[:, :, sl].rearrange("b p n -> p b n"),
                )
            # x-padding columns of the l2 input
            nc.vector.memset(pad_sb[:, :, :, 0:1], 0.0)
            nc.vector.memset(pad_sb[:, :, :, 65:66], 0.0)

            # ---- SE branch (tiny)
            nc.vector.tensor_tensor(
                pooled, yss_sb[:, 0:2], yss_sb[:, 2:4], AluOpType.add
            )
            nc.vector.tensor_scalar_mul(pooled, pooled, 1.0 / N)
            pr = ps.tile([128, 1], F32, tag="tiny")
            _mm(nc, pr, r1w[:, 0, :], pooled[:, 0:1], True, False)
            _mm(nc, pr, r1w[:, 1, :], pooled[:, 1:2], False, True)
            nc.vector.tensor_scalar(
                yr1_sb, pr, r1b, 0.0, AluOpType.add, AluOpType.max
            )
            for cb in range(2):
                pr2 = ps.tile([128, 1], F32, tag="tiny")
                _mm(nc, pr2, r2w[:, bass.ts(cb, 128)], yr1_sb, True, True)
                nc.vector.tensor_scalar_add(
                    yr_sb[:, cb : cb + 1], pr2, r2b[:, cb : cb + 1]
                )

            # ---- l1 over the 34-row halo extent, output into the padded
            # fp8 tile consumed by l2
            row_chunks = [(0, 8), (8, 8), (16, 8), (24, 8), (32, 2)]
            for cb in range(2):
                for (r0, nr) in row_chunks:
                    pl1 = ps.tile([128, 8, 64], F32, tag="l1", bufs=2)
                    p1 = pl1[:, 0:nr, :]
                    _mm(nc, p1, l1w[:, 0, cb, :],
                        yh_sb[:, 0, r0 * 64 : (r0 + nr) * 64], True, False)
                    _mm(nc, p1, l1w[:, 1, cb, :],
                        yh_sb[:, 1, r0 * 64 : (r0 + nr) * 64], False, True)
                    nc.scalar.activation(
                        pad_sb[:, cb, r0 : r0 + nr, 1:65], p1, ACT.Relu,
                        bias=l1b[:, cb : cb + 1],
                    )
                    if r0 == 0:
                        nc.vector.tensor_scalar_mul(
                            pad_sb[:, cb, 0, :], pad_sb[:, cb, 0, :],
                            bm_sb[:, 0:1])
                    elif r0 == 32:
                        nc.vector.tensor_scalar_mul(
                            pad_sb[:, cb, 33, :], pad_sb[:, cb, 33, :],
                            bm_sb[:, 1:2])

            # ---- l2 (3x3 conv, fp8 DoubleRow over both ci chunks) -> l3 ->
            # final gate; j-major so l3 pipelines behind l2
            for j in range(4):
                sl = bass.ts(j, 512)
                yl2j = []
                for cb in range(2):
                    pl2 = ps.tile([128, 8, 64], F32, tag="l2", bufs=2)
                    for s in range(2):
                        for t in range(9):
                            ty, tx = divmod(t, 3)
                            _mm(
                                nc, pl2,
                                l2w[:, s, t, cb, :, :],
                                pad_sb[:, :, j * 8 + ty : j * 8 + ty + 8,
                                       tx : tx + 64],
                                s == 0 and t == 0, s == 1 and t == 8,
                                perf_mode=DR,
                            )
                    y2t = tmp.tile([128, 8, 64], BF16, tag=f"y2{cb}")
                    nc.scalar.activation(
                        y2t, pl2, ACT.Relu, bias=l2b[:, cb : cb + 1]
                    )
                    yl2j.append(y2t.rearrange("p a b -> p (a b)"))
                # the last chunk runs the post-PE tail (bias/relu -> sigmoid
                # -> gate -> store) in 256-wide slices to halve the drain
                subs = ((0, 512),) if j < 3 else ((0, 256), (256, 256))
                for cb in range(2):
                    for s0, sw in subs:
                        pl3 = ps.tile([128, 512], F32, tag="l3", bufs=2)
                        p3 = pl3[:, 0:sw]
                        _mm(nc, p3, l3w[:, 0, cb, :],
                            yl2j[0][:, s0 : s0 + sw], True, False)
                        _mm(nc, p3, l3w[:, 1, cb, :],
                            yl2j[1][:, s0 : s0 + sw], False, True)
                        ssl = slice(j * 512 + s0, j * 512 + s0 + sw)
                        y3 = tmp.tile([128, 512], F32, tag="y3")
                        y3w = y3[:, 0:sw]
                        nc.vector.tensor_scalar(
                            y3w, p3, l3b[:, cb : cb + 1], 0.0,
                            AluOpType.add, AluOpType.max,
                        )
                        gt = tmp.tile([128, 512], BF16, tag="g")
                        gtw = gt[:, 0:sw]
                        nc.scalar.activation(
                            gtw, y3w, ACT.Sigmoid, bias=yr_sb[:, cb : cb + 1]
                        )
                        ot = tmp.tile([128, 512], BF16, tag=f"o{cb}")
                        otw = ot[:, 0:sw]
                        nc.vector.tensor_tensor(
                            otw, gtw, feat_sb[:, cb, ssl], AluOpType.mult
                        )
                        nc.sync.dma_start(out=out_d[cb][:, ssl], in_=otw)

    _split_multi_waits(nc)
    return nc


# ---------------------------------------------------------------- host side

_CACHE = {}


def _programs():
    if "a" not in _CACHE:
        _CACHE["a"] = _build_phase_a()
        _CACHE["b"] = _build_phase_b()
    return _CACHE["a"], _CACHE["b"]


def _fold_bn(w, bias, g, bb, m, v):
    s = g / np.sqrt(v + EPS)
    t = bb - s * m
    wf = w * (s[:, None] if w.ndim == 2 else s[:, None, None, None])
    return wf.astype(np.float32), (s * bias + t).astype(np.float32)


def _bf16(a):
    import ml_dtypes
    return np.ascontiguousarray(a).astype(ml_dtypes.bfloat16)


def _fp8(a):
    import ml_dtypes
    return np.ascontiguousarray(a).astype(ml_dtypes.float8_e4m3)


def _pm(w, groups):
    """[ci, m] -> [128, ci_chunks * m] per-partition pack (chunk-major)."""
    return np.ascontiguousarray(
        w.reshape(groups, 128, -1).transpose(1, 0, 2).reshape(128, -1)
    )


def _prep_weights(inp):
    def fold(wk2, bk2, pre):
        return _fold_bn(inp[wk2], inp[bk2], inp[pre + "_g"], inp[pre + "_b"],
                        inp[pre + "_m"], inp[pre + "_v"])

    d1w, d1b = fold("d1_w", "d1_b", "bn1")
    l1w, l1b = fold("l1_w", "l1_b", "lbn1")
    l2w, l2b = fold("l2_w", "l2_b", "lbn2")
    l3w, l3b = fold("l3_w", "l3_b", "lbn3")
    r1w, r1b = fold("r1_w", "r1_b", "rbn")

    f32v = lambda a: np.ascontiguousarray(np.asarray(a, np.float32))
    p = {}
    # phase A packs
    p["wf"] = np.concatenate(
        [_pm(inp["wq"].T, 2), _pm(inp["wk"].T, 2), _pm(inp["wv"].T, 2)], axis=1
    ).astype(np.float32)
    # d1w pack [128ci, 2ci, 2cb, 128co]: lhsT slices [ci, co]
    d1p = d1w.T.reshape(2, 128, 2, 128).transpose(1, 0, 2, 3).reshape(128, -1)
    p["wb"] = _bf16(np.concatenate(
        [d1p, np.eye(128, dtype=np.float32)], axis=1))
    # bias cols: bq, bv (2 chunks), d1bh = 0.5*d1b (feat already carries bv
    # when d1 reads it; the 0.5 feeds the tanh(z/2) gate formulation)
    d1bh = 0.5 * d1b
    p["wc"] = np.stack(
        [inp["bq"],
         inp["bv"][0:128], inp["bv"][128:256],
         d1bh[0:128], d1bh[128:256]], axis=1
    ).astype(np.float32)

    # phase B packs
    # l2 fp8 DoubleRow with a residual pass: w ~= fp8(w) + fp8(w - fp8(w)).
    # [co 256, ci 256, 3, 3] -> [ci_lo, pass 2, tap 9, cb 2, ci_hi 2, co 128]
    import ml_dtypes
    w2 = l2w.reshape(2, 128, 2, 128, 9)  # [cb, co, ci_hi, ci_lo, tap]
    w2_hi = w2.astype(ml_dtypes.float8_e4m3).astype(np.float32)
    w2_lo = (w2 - w2_hi).astype(ml_dtypes.float8_e4m3).astype(np.float32)
    l2p = np.ascontiguousarray(
        np.stack([w2_hi, w2_lo])  # [pass, cb, co, ci_hi, ci_lo, tap]
        .transpose(4, 0, 5, 1, 3, 2)  # [ci_lo, pass, tap, cb, ci_hi, co]
    ).reshape(128, -1)
    p["w8"] = _fp8(l2p)
    l1p = l1w.T.reshape(2, 128, 2, 128).transpose(1, 0, 2, 3).reshape(128, -1)
    l3p = l3w.T.reshape(2, 128, 2, 128).transpose(1, 0, 2, 3).reshape(128, -1)
    p["wbB"] = _bf16(np.concatenate([l1p, l3p], axis=1))
    p["wfB"] = np.concatenate(
        [
            np.stack([l1b[0:128], l1b[128:256], l2b[0:128], l2b[128:256],
                      l3b[0:128], l3b[128:256]], axis=1),
            _pm(r1w.T, 2),
            f32v(r1b[:, None]),
            f32v(inp["r2_w"].T),
            f32v(inp["r2_b"].reshape(2, 128).T),
        ],
        axis=1,
    ).astype(np.float32)
    return p


def _run_spmd(nc, in_maps):
    # transient NRT_EXEC_UNIT_UNRECOVERABLE wedges have been observed on
    # this fabric, and they poison the live PJRT client; reset the jax
    # backend between attempts so the retry lands on a clean client.
    import time as _time

    last = None
    for attempt in range(4):
        try:
            return run_bass_kernel_spmd(
                nc, in_maps, core_ids=list(range(NCORES))
            )
        except Exception as e:
            last = e
            try:
                import jax

                jax.clear_caches()
                jax.clear_backends()
            except Exception:
                pass
            _time.sleep(2.0 * (attempt + 1))
    raise last


def kernel(**inputs):
    nca, ncb = _programs()
    p = _prep_weights(inputs)
    x = inputs["x"].astype(np.float32).reshape(B, 2, 128, N)

    in_maps_a = []
    for core in range(NCORES):
        b, s = divmod(core, 2)
        m = {"wf": p["wf"], "wb": p["wb"], "wc": p["wc"]}
        if s == 0:
            m["xb"] = x[b]
        else:
            # rotate so this core's query half occupies columns [0, NH);
            # attention is permutation-invariant over key positions.
            m["xb"] = np.ascontiguousarray(
                np.concatenate([x[b][:, :, NH:], x[b][:, :, :NH]], axis=2)
            )
        in_maps_a.append(m)
    res_a = _run_spmd(nca, in_maps_a)
    feats = [np.asarray(r["feat"]) for r in res_a.results]
    ys = [np.asarray(r["y"]).reshape(2, 128, 32, 64) for r in res_a.results]
    ysums = [np.asarray(r["ysum"], np.float32) for r in res_a.results]

    zrow = np.zeros((2, 128, 1, 64), ys[0].dtype)
    in_maps_b = []
    for core in range(NCORES):
        b, s = divmod(core, 2)
        m = {"w8": p["w8"], "wb": p["wbB"], "wf": p["wfB"]}
        m["feat"] = feats[core]
        own, other = ys[core], ys[2 * b + (1 - s)]
        if s == 0:
            yhh = np.concatenate([zrow, own, other[:, :, 0:1]], axis=2)
        else:
            yhh = np.concatenate([other[:, :, 31:32], own, zrow], axis=2)
        m["yh"] = np.ascontiguousarray(yhh).reshape(2, 128, NHALO)
        m["yss"] = np.concatenate(
            [ysums[2 * b], ysums[2 * b + 1]], axis=1
        ).astype(np.float32)
        flags = (0.0, 1.0) if s == 0 else (1.0, 0.0)
        m["bm"] = np.tile(np.array(flags, np.float32), (128, 1))
        in_maps_b.append(m)
    res_b = _run_spmd(ncb, in_maps_b)

    out = np.empty((B, C, H, W), np.float32)
    for core in range(NCORES):
        b, s = divmod(core, 2)
        out[b, :, s * 32 : (s + 1) * 32, :] = (
            np.asarray(res_b.results[core]["out"], dtype=np.float32)
            .reshape(C, 32, 64)
        )
    return out

